# revision 9
# baseline (speedup 1.0000x reference)
"""Trainium2 Bass kernel for nn_MessageBlock (PaiNN-style GNN message passing).

Strategy (8 cores, destination-sharded):
- Nodes are split 2500/core. Each core processes exactly the edges whose
  i_index lands in its node slice (host buckets edges by dest 128-node block,
  pads each block's edge list to a uniform tile count -> identical SPMD
  program on every core, no cross-core scatter traffic).
- Phase D: dense per-node phi2 = silu(s@W1+b1)@W2+b2 for the core's own
  nodes (feature-major matmuls), cast fp16, AllGather -> phi2_all in DRAM.
- Phase E: per 128-edge subtile: dma_gather phi2_all[j] and v[j] (fp16),
  RBF filter x = [rbf;1]@[Wr;br] (K=21 matmul), cutoff gate
  W = 1 - Sin(x*pi/10)^2  (== 0.5*(cos(x*pi/5)+1); the x<5 mask is provably
  always true for rbf in [0,1] with |Wr| <= 1/sqrt(20)), payload build on
  DVE in fp16, scatter-add via one-hot matmul into per-block PSUM
  accumulators [ds | dv(d-major)] in fp32.
- Phase N: PE-transpose deltas to feature-major, v'=v+dv, s'=s+ds, then the
  node update MLP (Uv/Vv matmuls, |Vv| via Sqrt, gated update), transpose
  back to node-major and DMA out.
"""

import os
import sys

sys.path.insert(0, "/opt/trn_rl_repo")

import numpy as np

import concourse.bass as bass
import concourse.bacc as bacc
import concourse.mybir as mybir
from concourse.tile import TileContext

F16 = mybir.dt.float16
F32 = mybir.dt.float32
I16 = mybir.dt.int16

PI = float(np.pi)


class Cfg:
    def __init__(self, n_nodes=20000, n_edges=320000, n_cores=8, nrbf=20, f=128):
        self.N = n_nodes
        self.E = n_edges
        self.C = n_cores
        self.NRBF = nrbf
        self.F = f
        assert n_nodes % n_cores == 0
        self.NPC = n_nodes // n_cores            # nodes per core
        self.NBLK = (self.NPC + 127) // 128      # dest blocks per core
        # node-group size for phase D/N matmuls (<=512, divides work evenly)
        self.NG = min(500, self.NPC)

    def blk_rows(self, b):
        return min(128, self.NPC - b * 128)


CFG = Cfg()


# ----------------------------------------------------------------- host prep

def host_prep(inputs, cfg):
    """Bucket/pad edges, build per-core arrays. Returns (in_maps, meta)."""
    N, E, C, F, NRBF = cfg.N, cfg.E, cfg.C, cfg.F, cfg.NRBF
    NPC, NBLK = cfg.NPC, cfg.NBLK

    s = np.asarray(inputs["s"], np.float32)
    v = np.asarray(inputs["v"], np.float32)
    rbf = np.asarray(inputs["rbf"], np.float32)
    dirv = np.asarray(inputs["r_ij_direction"], np.float32)
    i_idx = np.asarray(inputs["i_index"]).astype(np.int64)
    j_idx = np.asarray(inputs["j_index"]).astype(np.int64)

    # global bucket: core, then 128-node block within core
    core_of = i_idx // NPC
    blk_of = (i_idx % NPC) // 128
    order = np.lexsort((blk_of, core_of))
    # per (core, block) counts
    counts = np.zeros((C, NBLK), np.int64)
    np.add.at(counts, (core_of, blk_of), 1)
    t_blk = max(1, int(np.max((counts + 127) // 128)))
    pblk = t_blk * 128                      # padded edges per block
    epad = NBLK * pblk                      # padded edges per core
    tt = NBLK * t_blk                       # subtiles per core

    # scatter edges into padded slots
    j_pad = np.zeros((C, epad), np.int64)
    iloc_pad = np.full((C, epad), 300.0, np.float32)
    rbf_pad = np.zeros((C, epad, NRBF), np.float32)
    dir_pad = np.zeros((C, epad, 3), np.float32)

    e_sorted = order
    cs = core_of[e_sorted]
    bs = blk_of[e_sorted]
    # position within (core, block)
    # lexsorted -> edges of each (c,b) contiguous; rank within group:
    grp = cs * NBLK + bs
    grp_start = np.searchsorted(grp, np.arange(C * NBLK), side="left")
    pos_in_grp = np.arange(E) - grp_start[grp]
    slot = bs * pblk + pos_in_grp
    j_pad[cs, slot] = j_idx[e_sorted]
    iloc_pad[cs, slot] = (i_idx[e_sorted] % NPC - bs * 128).astype(np.float32)
    rbf_pad[cs, slot] = rbf[e_sorted]
    dir_pad[cs, slot] = dirv[e_sorted]

    # weights / constants (identical per core)
    def f32(x):
        return np.ascontiguousarray(np.asarray(x, np.float32))

    W1, b1 = f32(inputs["W1"]), f32(inputs["b1"])
    W2, b2 = f32(inputs["W2"]), f32(inputs["b2"])
    Wr, br = f32(inputs["Wr"]), f32(inputs["br"])
    WU, bU = f32(inputs["WU"]), f32(inputs["bU"])
    WV, bV = f32(inputs["WV"]), f32(inputs["bV"])
    M1, bm1 = f32(inputs["M1"]), f32(inputs["bm1"])
    M2, bm2 = f32(inputs["M2"]), f32(inputs["bm2"])

    wr_aug = np.concatenate([Wr, br[None, :]], 0)          # [NRBF+1, 3F]
    ident = np.eye(128, dtype=np.float32)
    iota16 = np.broadcast_to(
        np.arange(128, dtype=np.float32).astype(np.float16), (128, 128)).copy()
    ones_row = np.ones((1, 128), np.float32)
    v_dm = np.ascontiguousarray(
        v.transpose((0, 2, 1)).reshape(N, 3 * F)).astype(np.float16)  # [N, 3F] d-major

    common = dict(
        w1=W1, b1c=b1[:, None], w2=W2, b2r=b2[None, :],
        wr_aug=wr_aug, ones_row=ones_row,
        wu=WU, buc=bU[:, None], wv=WV, bvc=bV[:, None],
        m1a=np.ascontiguousarray(M1[:F]), m1b=np.ascontiguousarray(M1[F:]),
        bm1c=bm1[:, None],
        m2=M2, bm2c=np.ascontiguousarray(bm2.reshape(3, F).T),  # [F, 3]
        ident=ident, iota16=iota16, v_dm=v_dm,
    )

    in_maps = []
    for c in range(C):
        lo, hi = c * NPC, (c + 1) * NPC
        jw = np.tile(
            np.ascontiguousarray(j_pad[c].astype(np.int16).reshape(-1, 16).T),
            (8, 1))                                         # [128, epad/16]
        m = dict(common)
        m.update(
            st_slice=np.ascontiguousarray(s[lo:hi].T),          # [F, NPC]
            v_fm=np.ascontiguousarray(v[lo:hi].transpose(2, 1, 0)),  # [3,F,NPC]->? see below
            jw=jw,
            iloc=np.ascontiguousarray(
                iloc_pad[c].reshape(tt, 128).T),                # [128, tt]
            dir_t=np.ascontiguousarray(
                dir_pad[c].reshape(tt, 128, 3).transpose(1, 0, 2)),  # [128, tt, 3]
            rbf_aug=np.ascontiguousarray(
                np.concatenate([rbf_pad[c],
                                np.ones((epad, 1), np.float32)], 1).T),  # [NRBF+1, epad]
        )
        # v_fm device layout [F, 3, NPC]
        m["v_fm"] = np.ascontiguousarray(v[lo:hi].transpose(1, 2, 0))
        in_maps.append(m)

    meta = dict(t_blk=t_blk, epad=epad, tt=tt)
    return in_maps, meta


# ------------------------------------------------------------- build program

def build_program(cfg, t_blk, debug=False):
    N, C, F, NRBF = cfg.N, cfg.C, cfg.F, cfg.NRBF
    NPC, NBLK, NG = cfg.NPC, cfg.NBLK, cfg.NG
    F3 = 3 * F
    pblk = t_blk * 128
    epad = NBLK * pblk
    tt = NBLK * t_blk

    nc = bacc.Bacc("TRN2", target_bir_lowering=False, num_devices=C)

    # ---- dram I/O
    d = {}
    def din(name, shape, dt=F32):
        d[name] = nc.dram_tensor(name, shape, dt, kind="ExternalInput")
        return d[name]

    din("w1", [F, F]); din("b1c", [F, 1])
    din("w2", [F, F3]); din("b2r", [1, F3])
    din("wr_aug", [NRBF + 1, F3]); din("ones_row", [1, 128])
    din("wu", [F, F]); din("buc", [F, 1])
    din("wv", [F, F]); din("bvc", [F, 1])
    din("m1a", [F, F]); din("m1b", [F, F]); din("bm1c", [F, 1])
    din("m2", [F, F3]); din("bm2c", [F, 3])
    din("ident", [128, 128]); din("iota16", [128, 128], F16)
    din("v_dm", [N, F3], F16)
    din("st_slice", [F, NPC]); din("v_fm", [F, 3, NPC])
    din("jw", [128, epad // 16], I16)
    din("iloc", [128, tt]); din("dir_t", [128, tt, 3])
    din("rbf_aug", [NRBF + 1, epad])

    d_sout = nc.dram_tensor("s_out", [NPC, F], F32, kind="ExternalOutput")
    d_vout = nc.dram_tensor("v_out", [NPC, F3], F32, kind="ExternalOutput")

    # collective tensors (internal dram)
    d_dbg_sp = d_dbg_vp = None
    if debug:
        d_dbg_sp = nc.dram_tensor("dbg_sp", [F, NPC], F32, kind="ExternalOutput")
        d_dbg_vp = nc.dram_tensor("dbg_vp", [F, 3, NPC], F32, kind="ExternalOutput")
        d_dbg_acc = nc.dram_tensor("dbg_acc", [128, NBLK, 512], F32, kind="ExternalOutput")
    d_phi2_slice = nc.dram_tensor("phi2_slice", [NPC, F3], F16, kind="Internal")
    d_phi2_all = nc.dram_tensor(
        "phi2_all", [N, F3], F16, kind="Internal", addr_space="Shared")

    AF = mybir.ActivationFunctionType
    OP = mybir.AluOpType

    with TileContext(nc) as tc:
        with tc.tile_pool(name="const", bufs=1) as cpool, \
             tc.tile_pool(name="spvp", bufs=1) as poolsv:
            t_ident = cpool.tile_from(d["ident"][:])
            t_sp = poolsv.tile([F, NPC], F32)          # s' feature-major
            t_vp = poolsv.tile([F, 3, NPC], F32)       # v' feature-major

            with tc.tile_pool(name="de", bufs=1) as poolde:
                t_st = poolde.tile_from(d["st_slice"][:])
                t_vfm = poolde.tile_from(d["v_fm"][:])
                t_acc = poolde.tile([128, NBLK, 512], F32)  # scatter accum

                # ---------------- Phase D: dense phi2 for own nodes
                with tc.tile_pool(name="phd", bufs=2) as pool, \
                     tc.tile_pool(name="phdc", bufs=1) as pdc, \
                     tc.tile_pool(name="phd_ps", bufs=2, space="PSUM") as psp:
                    t_w1 = pdc.tile_from(d["w1"][:])
                    t_b1 = pdc.tile_from(d["b1c"][:])
                    t_w2 = pdc.tile_from(d["w2"][:])
                    t_b2r = pdc.tile_from(d["b2r"][:])
                    t_ones = pdc.tile_from(d["ones_row"][:])
                    t_phi1 = pdc.tile([F, NPC], F32)
                    ngrp = (NPC + NG - 1) // NG
                    for g in range(ngrp):
                        n0 = g * NG
                        ns = min(NG, NPC - n0)
                        ps = psp.tile([128, 512], F32, tag="d1")
                        nc.tensor.matmul(ps[:, :ns], t_w1[:],
                                         t_st[:, n0:n0 + ns],
                                         start=True, stop=True)
                        nc.scalar.activation(t_phi1[:, n0:n0 + ns], ps[:, :ns],
                                             AF.Silu, bias=t_b1[:], scale=1.0)
                    for nb in range(NBLK):
                        n0 = nb * 128
                        ns = min(128, NPC - n0)
                        ps2 = psp.tile([128, F3], F32, tag="d2")
                        nc.tensor.matmul(ps2[:ns, :], t_phi1[:, n0:n0 + ns],
                                         t_w2[:], start=True, stop=False)
                        nc.tensor.matmul(ps2[:ns, :], t_ones[:1, :ns], t_b2r[:],
                                         start=False, stop=True)
                        t_p2 = pool.tile([128, F3], F16, tag="p2")
                        nc.scalar.activation(t_p2[:ns, :], ps2[:ns, :], AF.Copy)
                        nc.sync.dma_start(d_phi2_slice[n0:n0 + ns, :],
                                          t_p2[:ns, :])

                # AllGather phi2 slices -> phi2_all
                nc.gpsimd.collective_compute(
                    "AllGather", OP.bypass,
                    replica_groups=[list(range(C))],
                    ins=[d_phi2_slice[:, :]],
                    outs=[d_phi2_all[:, :]],
                )

                # ---------------- Phase E: edges
                GSZ = 4  # subtiles per group
                with tc.tile_pool(name="phe", bufs=2) as pool, \
                     tc.tile_pool(name="phec", bufs=1) as pec, \
                     tc.tile_pool(name="phe_x", bufs=1, space="PSUM") as pspx, \
                     tc.tile_pool(name="phe_a", bufs=2, space="PSUM") as pspa:
                    t_wr = pec.tile_from(d["wr_aug"][:])
                    t_iota = pec.tile_from(d["iota16"][:])
                    t_iloc = pec.tile_from(d["iloc"][:])
                    t_dirt = pec.tile_from(d["dir_t"][:])
                    t_jw = pec.tile_from(d["jw"][:])
                    for b in range(NBLK):
                        acc = pspa.tile([128, 512], F32, tag="acc")
                        st_base = b * t_blk
                        for g0 in range(0, t_blk, GSZ):
                            gsz = min(GSZ, t_blk - g0)
                            st0 = st_base + g0          # global subtile idx
                            ne = gsz * 128
                            e0 = st0 * 128
                            t_phi = pool.tile([128, GSZ, F3], F16, tag="gphi")
                            nc.gpsimd.dma_gather(
                                t_phi[:, :gsz, :], d_phi2_all[:, :],
                                t_jw[:, st0 * 8: st0 * 8 + gsz * 8],
                                ne, ne, F3)
                            t_vj = pool.tile([128, GSZ, F3], F16, tag="gvj")
                            nc.gpsimd.dma_gather(
                                t_vj[:, :gsz, :], d["v_dm"][:, :],
                                t_jw[:, st0 * 8: st0 * 8 + gsz * 8],
                                ne, ne, F3)
                            t_rbf = pool.tile([NRBF + 1, GSZ * 128], F32,
                                              tag="rbf")
                            nc.sync.dma_start(
                                t_rbf[:, :ne], d["rbf_aug"][:, e0:e0 + ne])
                            xps = pspx.tile([128, GSZ, 512], F32, tag="xps")
                            for si in range(gsz):
                                nc.tensor.matmul(
                                    xps[:, si, 0:F3],
                                    t_rbf[:, si * 128:(si + 1) * 128],
                                    t_wr[:], start=True, stop=True)
                            # gate: W = 1 - sin(x*pi/10)^2
                            t_u = pool.tile([128, GSZ, F3], F16, tag="u")
                            nc.scalar.activation(t_u[:, :gsz, :],
                                                 xps[:, :gsz, 0:F3],
                                                 AF.Sin, bias=0.0,
                                                 scale=PI / 10)
                            t_u2 = pool.tile([128, GSZ, F3], F16, tag="u2")
                            nc.scalar.activation(t_u2[:, :gsz, :],
                                                 t_u[:, :gsz, :], AF.Square)
                            t_g = pool.tile([128, GSZ, F3], F16, tag="g")
                            nc.vector.tensor_scalar(
                                t_g[:, :gsz, :], t_u2[:, :gsz, :], -1.0, 1.0,
                                OP.mult, OP.add)
                            t_split = pool.tile([128, GSZ, F3], F16,
                                                tag="split")
                            nc.vector.tensor_tensor(
                                t_split[:, :gsz, :], t_phi[:, :gsz, :],
                                t_g[:, :gsz, :], OP.mult)
                            t_dvp = pool.tile([128, GSZ, 3, F], F16, tag="dvp")
                            nc.vector.tensor_tensor(
                                t_dvp[:, :gsz, :, :],
                                t_split[:, :gsz, 2 * F:3 * F].unsqueeze(2)
                                    .to_broadcast((128, gsz, 3, F)),
                                t_vj[:, :gsz, :].rearrange(
                                    "p s (d f) -> p s d f", d=3),
                                OP.mult)
                            t_oh = pool.tile([128, GSZ, 128], F16, tag="oh")
                            t_rd = pool.tile([128, GSZ, 3, F], F16, tag="rd")
                            for si in range(gsz):
                                nc.vector.tensor_scalar(
                                    t_oh[:, si, :], t_iota[:],
                                    t_iloc[:, st0 + si: st0 + si + 1], None,
                                    OP.is_equal)
                                for dd in range(3):
                                    nc.vector.tensor_scalar(
                                        t_rd[:, si, dd, :],
                                        t_split[:, si, F:2 * F],
                                        t_dirt[:, st0 + si, dd:dd + 1], None,
                                        OP.mult)
                            for si in range(gsz):
                                first = (g0 == 0 and si == 0)
                                last = (g0 + gsz == t_blk and si == gsz - 1)
                                nc.tensor.matmul(
                                    acc[:, 0:F], t_oh[:, si, :],
                                    t_split[:, si, 0:F],
                                    start=first, stop=last,
                                    skip_group_check=True)
                                nc.tensor.matmul(
                                    acc[:, F:512], t_oh[:, si, :],
                                    t_dvp[:, si, :, :].rearrange(
                                        "p d f -> p (d f)"),
                                    start=False, stop=False,
                                    skip_group_check=True)
                                nc.tensor.matmul(
                                    acc[:, F:512], t_oh[:, si, :],
                                    t_rd[:, si, :, :].rearrange(
                                        "p d f -> p (d f)"),
                                    start=False, stop=last,
                                    skip_group_check=True)
                        nc.vector.tensor_copy(t_acc[:, b, :], acc[:])

                # ---------------- Phase N-A: transpose deltas, add bases
                with tc.tile_pool(name="phna_ps", bufs=2,
                                  space="PSUM") as psp:
                    for b in range(NBLK):
                        n0 = b * 128
                        ns = cfg.blk_rows(b)
                        tp = psp.tile([128, 512], F32, tag="tp")
                        for k in range(4):
                            nc.tensor.transpose(
                                tp[:, k * 128:(k + 1) * 128],
                                t_acc[:, b, k * 128:(k + 1) * 128],
                                t_ident[:])
                        nc.vector.tensor_tensor(
                            t_sp[:, n0:n0 + ns], tp[:, 0:ns],
                            t_st[:, n0:n0 + ns], OP.add)
                        nc.vector.tensor_tensor(
                            t_vp[:, :, n0:n0 + ns],
                            tp[:, 128:512].rearrange(
                                "p (d n) -> p d n", d=3)[:, :, 0:ns],
                            t_vfm[:, :, n0:n0 + ns], OP.add)
                    if debug:
                        nc.sync.dma_start(d_dbg_sp[:, :], t_sp[:])
                        nc.sync.dma_start(d_dbg_vp[:, :, :], t_vp[:])
                        nc.sync.dma_start(d_dbg_acc[:, :, :], t_acc[:])

            # ---------------- Phase N-B..H (st/vfm/acc freed)
            with tc.tile_pool(name="phn", bufs=2) as pool, \
                 tc.tile_pool(name="phnc", bufs=1) as pnc, \
                 tc.tile_pool(name="phn_ps", bufs=2, space="PSUM") as psp:
                t_wu = pnc.tile_from(d["wu"][:])
                t_bu = pnc.tile_from(d["buc"][:])
                t_wv = pnc.tile_from(d["wv"][:])
                t_bv = pnc.tile_from(d["bvc"][:])
                t_m1a = pnc.tile_from(d["m1a"][:])
                t_m1b = pnc.tile_from(d["m1b"][:])
                t_bm1 = pnc.tile_from(d["bm1c"][:])
                t_m2 = pnc.tile_from(d["m2"][:])
                t_bm2 = pnc.tile_from(d["bm2c"][:])
                t_uv = pnc.tile([F, 3, NPC], F32)
                t_vnorm = pnc.tile([F, NPC], F32)
                t_uvs = pnc.tile([F, NPC], F32)
                t_h = pnc.tile([F, NPC], F32)
                t_m = pnc.tile([F, 3, NPC], F32)
                ngrp = (NPC + NG - 1) // NG
                for g in range(ngrp):
                    n0 = g * NG
                    ns = min(NG, NPC - n0)
                    t_vv = pool.tile([F, 3, NG], F32, tag="vvg")
                    for dd in range(3):
                        psu = psp.tile([128, 512], F32, tag="u3")
                        nc.tensor.matmul(psu[:, :ns], t_wu[:],
                                         t_vp[:, dd, n0:n0 + ns],
                                         start=True, stop=True)
                        nc.scalar.activation(
                            t_uv[:, dd, n0:n0 + ns], psu[:, :ns],
                            AF.Identity, bias=t_bu[:], scale=1.0)
                        psvv = psp.tile([128, 512], F32, tag="u3")
                        nc.tensor.matmul(psvv[:, :ns], t_wv[:],
                                         t_vp[:, dd, n0:n0 + ns],
                                         start=True, stop=True)
                        nc.scalar.activation(
                            t_vv[:, dd, :ns], psvv[:, :ns],
                            AF.Identity, bias=t_bv[:], scale=1.0)
                    # vn2 = sum_d Vv^2 ; uvs = sum_d Uv*Vv
                    t_sq = pool.tile([F, 3, NG], F32, tag="sqg")
                    nc.vector.tensor_tensor(t_sq[:, :, :ns], t_vv[:, :, :ns],
                                            t_vv[:, :, :ns], OP.mult)
                    t_vn2 = pool.tile([F, NG], F32, tag="vn2")
                    nc.vector.tensor_reduce(
                        t_vn2[:, :ns],
                        t_sq[:, :, :ns].transpose((0, 2, 1)),
                        mybir.AxisListType.X, OP.add)
                    nc.vector.tensor_tensor(t_sq[:, :, :ns],
                                            t_uv[:, :, n0:n0 + ns],
                                            t_vv[:, :, :ns], OP.mult)
                    nc.vector.tensor_reduce(
                        t_uvs[:, n0:n0 + ns],
                        t_sq[:, :, :ns].transpose((0, 2, 1)),
                        mybir.AxisListType.X, OP.add)
                    nc.scalar.activation(t_vnorm[:, n0:n0 + ns],
                                         t_vn2[:, :ns], AF.Sqrt)
                    # h = silu(M1a@vnorm + M1b@s' + bm1)
                    psh = psp.tile([128, 512], F32, tag="h")
                    nc.tensor.matmul(psh[:, :ns], t_m1a[:],
                                     t_vnorm[:, n0:n0 + ns],
                                     start=True, stop=False)
                    nc.tensor.matmul(psh[:, :ns], t_m1b[:],
                                     t_sp[:, n0:n0 + ns],
                                     start=False, stop=True)
                    nc.scalar.activation(t_h[:, n0:n0 + ns], psh[:, :ns],
                                         AF.Silu, bias=t_bm1[:], scale=1.0)
                    # m = h@M2 + bm2  (3 chunks: avv, asv, ass)
                    for k in range(3):
                        psm = psp.tile([128, 512], F32, tag="m")
                        nc.tensor.matmul(psm[:, :ns],
                                         t_m2[:, k * F:(k + 1) * F],
                                         t_h[:, n0:n0 + ns],
                                         start=True, stop=True)
                        nc.scalar.activation(
                            t_m[:, k, n0:n0 + ns], psm[:, :ns],
                            AF.Identity, bias=t_bm2[:, k:k + 1], scale=1.0)

                # N-G: final combines
                nc.vector.tensor_tensor(
                    t_uv[:], t_uv[:],
                    t_m[:, 0:1, :].to_broadcast((F, 3, NPC)), OP.mult)
                nc.vector.tensor_tensor(t_vp[:], t_vp[:], t_uv[:], OP.add)
                nc.vector.tensor_tensor(t_uvs[:], t_uvs[:], t_m[:, 1, :],
                                        OP.mult)
                nc.vector.tensor_tensor(t_uvs[:], t_uvs[:], t_m[:, 2, :],
                                        OP.add)
                nc.vector.tensor_tensor(t_sp[:], t_sp[:], t_uvs[:], OP.add)

                # N-H: transpose back to node-major + DMA out
                for b in range(NBLK):
                    n0 = b * 128
                    ns = cfg.blk_rows(b)
                    tpo = psp.tile([128, 512], F32, tag="tp")
                    nc.tensor.transpose(tpo[:ns, 0:128],
                                        t_sp[:, n0:n0 + ns], t_ident[:])
                    for dd in range(3):
                        nc.tensor.transpose(
                            tpo[:ns, 128 + dd * 128:128 + (dd + 1) * 128],
                            t_vp[:, dd, n0:n0 + ns], t_ident[:])
                    t_so = pool.tile([128, F], F32, tag="so")
                    nc.scalar.activation(t_so[:ns, :], tpo[:ns, 0:F], AF.Copy)
                    nc.sync.dma_start(d_sout[n0:n0 + ns, :], t_so[:ns, :])
                    t_vo = pool.tile([128, F3], F32, tag="vo")
                    nc.vector.tensor_copy(
                        t_vo[:ns, :].rearrange("p (f d) -> p d f", d=3),
                        tpo[:ns, 128:512].rearrange("p (d f) -> p d f", d=3))
                    nc.sync.dma_start(d_vout[n0:n0 + ns, :], t_vo[:ns, :])

    nc.compile()
    return nc


# ------------------------------------------------------------------- runner

_CACHE = {}
LAST_RESULTS = None


def run(inputs, cfg=CFG, sim=False, trace=False):
    global LAST_RESULTS
    in_maps, meta = host_prep(inputs, cfg)
    key = (cfg.N, cfg.E, cfg.C, meta["t_blk"])
    if key not in _CACHE:
        _CACHE[key] = build_program(cfg, meta["t_blk"])
    nc = _CACHE[key]

    if sim:
        from concourse.bass_interp import MultiCoreSim
        ms = MultiCoreSim(nc, num_cores=cfg.C, require_finite=False,
                          require_nnan=False)
        sims = list(ms.cores.values())
        for c, m in enumerate(in_maps):
            for k2, v2 in m.items():
                sims[c].tensor(k2)[:] = v2
        ms.simulate(check_with_hw=False)
        outs = [{"s_out": np.asarray(s_.tensor("s_out")),
                 "v_out": np.asarray(s_.tensor("v_out"))} for s_ in sims]
    else:
        from concourse.bass_utils import run_bass_kernel_spmd
        res = run_bass_kernel_spmd(nc, in_maps, core_ids=list(range(cfg.C)),
                                   trace=trace)
        LAST_RESULTS = res
        outs = res.results

    s_out = np.concatenate([o["s_out"] for o in outs], 0)
    v_out = np.concatenate([o["v_out"] for o in outs], 0)
    return s_out, v_out.reshape(cfg.N, cfg.F, 3)


def kernel(**inputs):
    s_out, v_out = run(inputs, CFG, sim=False,
                       trace=bool(int(os.environ.get("KERNEL_TRACE", "0"))))
    return (s_out, v_out)


# revision 10
# speedup vs baseline: 79.3555x; 79.3555x over previous
"""Trainium2 Bass kernel for nn_MessageBlock (PaiNN-style GNN message passing).

Strategy (8 cores, destination-sharded):
- Nodes are split 2500/core. Each core processes exactly the edges whose
  i_index lands in its node slice (host buckets edges by dest 128-node block,
  pads each block's edge list to a uniform tile count -> identical SPMD
  program on every core, no cross-core scatter traffic).
- Phase D: dense per-node phi2 = silu(s@W1+b1)@W2+b2 for the core's own
  nodes (feature-major matmuls), cast fp16, AllGather -> phi2_all in DRAM.
- Phase E: per 128-edge subtile: dma_gather phi2_all[j] and v[j] (fp16),
  RBF filter x = [rbf;1]@[Wr;br] (K=21 matmul), cutoff gate
  W = 1 - Sin(x*pi/10)^2  (== 0.5*(cos(x*pi/5)+1); the x<5 mask is provably
  always true for rbf in [0,1] with |Wr| <= 1/sqrt(20)), payload build on
  DVE in fp16, scatter-add via one-hot matmul into per-block PSUM
  accumulators [ds | dv(d-major)] in fp32.
- Phase N: PE-transpose deltas to feature-major, v'=v+dv, s'=s+ds, then the
  node update MLP (Uv/Vv matmuls, |Vv| via Sqrt, gated update), transpose
  back to node-major and DMA out.
"""

import os
import sys

sys.path.insert(0, "/opt/trn_rl_repo")

import numpy as np

import concourse.bass as bass
import concourse.bacc as bacc
import concourse.mybir as mybir
from concourse.tile import TileContext

F16 = mybir.dt.float16
F32 = mybir.dt.float32
I16 = mybir.dt.int16

PI = float(np.pi)


class Cfg:
    def __init__(self, n_nodes=20000, n_edges=320000, n_cores=8, nrbf=20, f=128):
        self.N = n_nodes
        self.E = n_edges
        self.C = n_cores
        self.NRBF = nrbf
        self.F = f
        assert n_nodes % n_cores == 0
        self.NPC = n_nodes // n_cores            # nodes per core
        self.NBLK = (self.NPC + 127) // 128      # dest blocks per core
        # node-group size for phase D/N matmuls (<=512, divides work evenly)
        self.NG = min(500, self.NPC)

    def blk_rows(self, b):
        return min(128, self.NPC - b * 128)


CFG = Cfg()


# ----------------------------------------------------------------- host prep

def host_prep(inputs, cfg):
    """Bucket/pad edges, build per-core arrays. Returns (in_maps, meta)."""
    N, E, C, F, NRBF = cfg.N, cfg.E, cfg.C, cfg.F, cfg.NRBF
    NPC, NBLK = cfg.NPC, cfg.NBLK

    s = np.asarray(inputs["s"], np.float32)
    v = np.asarray(inputs["v"], np.float32)
    rbf = np.asarray(inputs["rbf"], np.float32)
    dirv = np.asarray(inputs["r_ij_direction"], np.float32)
    i_idx = np.asarray(inputs["i_index"]).astype(np.int64)
    j_idx = np.asarray(inputs["j_index"]).astype(np.int64)

    # global bucket: core, then 128-node block within core
    core_of = i_idx // NPC
    blk_of = (i_idx % NPC) // 128
    order = np.lexsort((blk_of, core_of))
    # per (core, block) counts
    counts = np.zeros((C, NBLK), np.int64)
    np.add.at(counts, (core_of, blk_of), 1)
    t_blk = max(1, int(np.max((counts + 127) // 128)))
    pblk = t_blk * 128                      # padded edges per block
    epad = NBLK * pblk                      # padded edges per core
    tt = NBLK * t_blk                       # subtiles per core

    # scatter edges into padded slots
    j_pad = np.zeros((C, epad), np.int64)
    iloc_pad = np.full((C, epad), 300.0, np.float32)
    rbf_pad = np.zeros((C, epad, NRBF), np.float32)
    dir_pad = np.zeros((C, epad, 3), np.float32)

    e_sorted = order
    cs = core_of[e_sorted]
    bs = blk_of[e_sorted]
    # position within (core, block)
    # lexsorted -> edges of each (c,b) contiguous; rank within group:
    grp = cs * NBLK + bs
    grp_start = np.searchsorted(grp, np.arange(C * NBLK), side="left")
    pos_in_grp = np.arange(E) - grp_start[grp]
    slot = bs * pblk + pos_in_grp
    j_pad[cs, slot] = j_idx[e_sorted]
    iloc_pad[cs, slot] = (i_idx[e_sorted] % NPC - bs * 128).astype(np.float32)
    rbf_pad[cs, slot] = rbf[e_sorted]
    dir_pad[cs, slot] = dirv[e_sorted]

    # weights / constants (identical per core)
    def f32(x):
        return np.ascontiguousarray(np.asarray(x, np.float32))

    W1, b1 = f32(inputs["W1"]), f32(inputs["b1"])
    W2, b2 = f32(inputs["W2"]), f32(inputs["b2"])
    Wr, br = f32(inputs["Wr"]), f32(inputs["br"])
    WU, bU = f32(inputs["WU"]), f32(inputs["bU"])
    WV, bV = f32(inputs["WV"]), f32(inputs["bV"])
    M1, bm1 = f32(inputs["M1"]), f32(inputs["bm1"])
    M2, bm2 = f32(inputs["M2"]), f32(inputs["bm2"])

    wr_aug = np.concatenate([Wr, br[None, :]], 0)          # [NRBF+1, 3F]
    ident = np.eye(128, dtype=np.float32)
    iota16 = np.broadcast_to(
        np.arange(128, dtype=np.float32).astype(np.float16), (128, 128)).copy()
    ones_row = np.ones((1, 128), np.float32)
    v_dm = np.ascontiguousarray(
        v.transpose((0, 2, 1)).reshape(N, 3 * F)).astype(np.float16)  # [N, 3F] d-major

    common = dict(
        w1=W1, b1c=b1[:, None], w2=W2, b2r=b2[None, :],
        wr_aug=wr_aug, ones_row=ones_row,
        wu=WU, buc=bU[:, None], wv=WV, bvc=bV[:, None],
        m1a=np.ascontiguousarray(M1[:F]), m1b=np.ascontiguousarray(M1[F:]),
        bm1c=bm1[:, None],
        m2=M2, bm2c=np.ascontiguousarray(bm2.reshape(3, F).T),  # [F, 3]
        ident=ident, iota16=iota16, v_dm=v_dm,
    )

    in_maps = []
    for c in range(C):
        lo, hi = c * NPC, (c + 1) * NPC
        jw = np.tile(
            np.ascontiguousarray(j_pad[c].astype(np.int16).reshape(-1, 16).T),
            (8, 1))                                         # [128, epad/16]
        m = dict(common)
        m.update(
            st_slice=np.ascontiguousarray(s[lo:hi].T),          # [F, NPC]
            v_fm=np.ascontiguousarray(v[lo:hi].transpose(2, 1, 0)),  # [3,F,NPC]->? see below
            jw=jw,
            iloc=np.ascontiguousarray(
                iloc_pad[c].reshape(tt, 128).T),                # [128, tt]
            dir_t=np.ascontiguousarray(
                dir_pad[c].reshape(tt, 128, 3).transpose(1, 0, 2)),  # [128, tt, 3]
            rbf_aug=np.ascontiguousarray(
                np.concatenate([rbf_pad[c],
                                np.ones((epad, 1), np.float32)], 1).T),  # [NRBF+1, epad]
        )
        # v_fm device layout [F, 3, NPC]
        m["v_fm"] = np.ascontiguousarray(v[lo:hi].transpose(1, 2, 0))
        in_maps.append(m)

    meta = dict(t_blk=t_blk, epad=epad, tt=tt)
    return in_maps, meta


# ------------------------------------------------------------- build program

def build_program(cfg, t_blk, debug=False):
    N, C, F, NRBF = cfg.N, cfg.C, cfg.F, cfg.NRBF
    NPC, NBLK, NG = cfg.NPC, cfg.NBLK, cfg.NG
    F3 = 3 * F
    pblk = t_blk * 128
    epad = NBLK * pblk
    tt = NBLK * t_blk

    nc = bacc.Bacc("TRN2", target_bir_lowering=False, num_devices=C)

    # ---- dram I/O
    d = {}
    def din(name, shape, dt=F32):
        d[name] = nc.dram_tensor(name, shape, dt, kind="ExternalInput")
        return d[name]

    din("w1", [F, F]); din("b1c", [F, 1])
    din("w2", [F, F3]); din("b2r", [1, F3])
    din("wr_aug", [NRBF + 1, F3]); din("ones_row", [1, 128])
    din("wu", [F, F]); din("buc", [F, 1])
    din("wv", [F, F]); din("bvc", [F, 1])
    din("m1a", [F, F]); din("m1b", [F, F]); din("bm1c", [F, 1])
    din("m2", [F, F3]); din("bm2c", [F, 3])
    din("ident", [128, 128]); din("iota16", [128, 128], F16)
    din("v_dm", [N, F3], F16)
    din("st_slice", [F, NPC]); din("v_fm", [F, 3, NPC])
    din("jw", [128, epad // 16], I16)
    din("iloc", [128, tt]); din("dir_t", [128, tt, 3])
    din("rbf_aug", [NRBF + 1, epad])

    d_sout = nc.dram_tensor("s_out", [NPC, F], F32, kind="ExternalOutput")
    d_vout = nc.dram_tensor("v_out", [NPC, F3], F32, kind="ExternalOutput")

    # collective tensors (internal dram)
    d_dbg_sp = d_dbg_vp = None
    if debug:
        d_dbg_sp = nc.dram_tensor("dbg_sp", [F, NPC], F32, kind="ExternalOutput")
        d_dbg_vp = nc.dram_tensor("dbg_vp", [F, 3, NPC], F32, kind="ExternalOutput")
        d_dbg_acc = nc.dram_tensor("dbg_acc", [128, NBLK, 512], F32, kind="ExternalOutput")
    d_phi2_slice = nc.dram_tensor("phi2_slice", [NPC, F3], F16, kind="Internal")
    d_phi2_all = nc.dram_tensor(
        "phi2_all", [N, F3], F16, kind="Internal", addr_space="Shared")

    AF = mybir.ActivationFunctionType
    OP = mybir.AluOpType

    with TileContext(nc) as tc:
        with tc.tile_pool(name="const", bufs=1) as cpool, \
             tc.tile_pool(name="spvp", bufs=1) as poolsv:
            t_ident = cpool.tile_from(d["ident"][:])
            t_sp = poolsv.tile([F, NPC], F32)          # s' feature-major
            t_vp = poolsv.tile([F, 3, NPC], F32)       # v' feature-major

            with tc.tile_pool(name="de", bufs=1) as poolde:
                t_st = poolde.tile_from(d["st_slice"][:])
                t_vfm = poolde.tile_from(d["v_fm"][:])
                t_acc = poolde.tile([128, NBLK, 512], F32)  # scatter accum

                # ---------------- Phase D: dense phi2 for own nodes
                with tc.tile_pool(name="phd", bufs=2) as pool, \
                     tc.tile_pool(name="phdc", bufs=1) as pdc, \
                     tc.tile_pool(name="phd_ps", bufs=2, space="PSUM") as psp:
                    t_w1 = pdc.tile_from(d["w1"][:])
                    t_b1 = pdc.tile_from(d["b1c"][:])
                    t_w2 = pdc.tile_from(d["w2"][:])
                    t_b2r = pdc.tile_from(d["b2r"][:])
                    t_ones = pdc.tile_from(d["ones_row"][:])
                    t_phi1 = pdc.tile([F, NPC], F32)
                    ngrp = (NPC + NG - 1) // NG
                    for g in range(ngrp):
                        n0 = g * NG
                        ns = min(NG, NPC - n0)
                        ps = psp.tile([128, 512], F32, tag="d1")
                        nc.tensor.matmul(ps[:, :ns], t_w1[:],
                                         t_st[:, n0:n0 + ns],
                                         start=True, stop=True)
                        nc.scalar.activation(t_phi1[:, n0:n0 + ns], ps[:, :ns],
                                             AF.Silu, bias=t_b1[:], scale=1.0)
                    for nb in range(NBLK):
                        n0 = nb * 128
                        ns = min(128, NPC - n0)
                        ps2 = psp.tile([128, F3], F32, tag="d2")
                        nc.tensor.matmul(ps2[:ns, :], t_phi1[:, n0:n0 + ns],
                                         t_w2[:], start=True, stop=False)
                        nc.tensor.matmul(ps2[:ns, :], t_ones[:1, :ns], t_b2r[:],
                                         start=False, stop=True)
                        t_p2 = pool.tile([128, F3], F16, tag="p2")
                        nc.scalar.activation(t_p2[:ns, :], ps2[:ns, :], AF.Copy)
                        nc.sync.dma_start(d_phi2_slice[n0:n0 + ns, :],
                                          t_p2[:ns, :])

                # AllGather phi2 slices -> phi2_all
                nc.gpsimd.collective_compute(
                    "AllGather", OP.bypass,
                    replica_groups=[list(range(C))],
                    ins=[d_phi2_slice[:, :]],
                    outs=[d_phi2_all[:, :]],
                )

                # ---------------- Phase E: edges
                GSZ = 4  # subtiles per group
                with tc.tile_pool(name="phe", bufs=2) as pool, \
                     tc.tile_pool(name="phec", bufs=1) as pec, \
                     tc.tile_pool(name="phe_x", bufs=1, space="PSUM") as pspx, \
                     tc.tile_pool(name="phe_a", bufs=2, space="PSUM") as pspa:
                    t_wr = pec.tile_from(d["wr_aug"][:])
                    t_iota = pec.tile_from(d["iota16"][:])
                    t_iloc = pec.tile_from(d["iloc"][:])
                    t_dirt = pec.tile_from(d["dir_t"][:])
                    t_jw = pec.tile_from(d["jw"][:])
                    for b in range(NBLK):
                        acc = pspa.tile([128, 512], F32, tag="acc")
                        st_base = b * t_blk
                        for g0 in range(0, t_blk, GSZ):
                            gsz = min(GSZ, t_blk - g0)
                            st0 = st_base + g0          # global subtile idx
                            ne = gsz * 128
                            e0 = st0 * 128
                            t_phi = pool.tile([128, GSZ, F3], F16, tag="gphi")
                            nc.gpsimd.dma_gather(
                                t_phi[:, :gsz, :], d_phi2_all[:, :],
                                t_jw[:, st0 * 8: st0 * 8 + gsz * 8],
                                ne, ne, F3)
                            t_vj = pool.tile([128, GSZ, F3], F16, tag="gvj")
                            nc.gpsimd.dma_gather(
                                t_vj[:, :gsz, :], d["v_dm"][:, :],
                                t_jw[:, st0 * 8: st0 * 8 + gsz * 8],
                                ne, ne, F3)
                            t_rbf = pool.tile([NRBF + 1, GSZ * 128], F32,
                                              tag="rbf")
                            nc.sync.dma_start(
                                t_rbf[:, :ne], d["rbf_aug"][:, e0:e0 + ne])
                            xps = pspx.tile([128, GSZ, 512], F32, tag="xps")
                            for si in range(gsz):
                                nc.tensor.matmul(
                                    xps[:, si, 0:F3],
                                    t_rbf[:, si * 128:(si + 1) * 128],
                                    t_wr[:], start=True, stop=True)
                            # gate: W = 1 - sin(x*pi/10)^2
                            t_u = pool.tile([128, GSZ, F3], F16, tag="u")
                            nc.scalar.activation(t_u[:, :gsz, :],
                                                 xps[:, :gsz, 0:F3],
                                                 AF.Sin, bias=0.0,
                                                 scale=PI / 10)
                            t_u2 = pool.tile([128, GSZ, F3], F16, tag="u2")
                            nc.scalar.activation(t_u2[:, :gsz, :],
                                                 t_u[:, :gsz, :], AF.Square)
                            t_g = pool.tile([128, GSZ, F3], F16, tag="g")
                            nc.vector.tensor_scalar(
                                t_g[:, :gsz, :], t_u2[:, :gsz, :], -1.0, 1.0,
                                OP.mult, OP.add)
                            t_split = pool.tile([128, GSZ, F3], F16,
                                                tag="split")
                            nc.vector.tensor_tensor(
                                t_split[:, :gsz, :], t_phi[:, :gsz, :],
                                t_g[:, :gsz, :], OP.mult)
                            t_dvp = pool.tile([128, GSZ, 3, F], F16, tag="dvp")
                            nc.vector.tensor_tensor(
                                t_dvp[:, :gsz, :, :],
                                t_split[:, :gsz, 2 * F:3 * F].unsqueeze(2)
                                    .to_broadcast((128, gsz, 3, F)),
                                t_vj[:, :gsz, :].rearrange(
                                    "p s (d f) -> p s d f", d=3),
                                OP.mult)
                            t_oh = pool.tile([128, GSZ, 128], F16, tag="oh")
                            t_rd = pool.tile([128, GSZ, 3, F], F16, tag="rd")
                            for si in range(gsz):
                                nc.vector.tensor_scalar(
                                    t_oh[:, si, :], t_iota[:],
                                    t_iloc[:, st0 + si: st0 + si + 1], None,
                                    OP.is_equal)
                                for dd in range(3):
                                    nc.vector.tensor_scalar(
                                        t_rd[:, si, dd, :],
                                        t_split[:, si, F:2 * F],
                                        t_dirt[:, st0 + si, dd:dd + 1], None,
                                        OP.mult)
                            for si in range(gsz):
                                first = (g0 == 0 and si == 0)
                                last = (g0 + gsz == t_blk and si == gsz - 1)
                                nc.tensor.matmul(
                                    acc[:, 0:F], t_oh[:, si, :],
                                    t_split[:, si, 0:F],
                                    start=first, stop=last,
                                    skip_group_check=True)
                                nc.tensor.matmul(
                                    acc[:, F:512], t_oh[:, si, :],
                                    t_dvp[:, si, :, :].rearrange(
                                        "p d f -> p (d f)"),
                                    start=False, stop=False,
                                    skip_group_check=True)
                                nc.tensor.matmul(
                                    acc[:, F:512], t_oh[:, si, :],
                                    t_rd[:, si, :, :].rearrange(
                                        "p d f -> p (d f)"),
                                    start=False, stop=last,
                                    skip_group_check=True)
                        nc.vector.tensor_copy(t_acc[:, b, :], acc[:])

                # ---------------- Phase N-A: transpose deltas, add bases
                with tc.tile_pool(name="phna_ps", bufs=2,
                                  space="PSUM") as psp:
                    for b in range(NBLK):
                        n0 = b * 128
                        ns = cfg.blk_rows(b)
                        tp = psp.tile([128, 512], F32, tag="tp")
                        for k in range(4):
                            nc.tensor.transpose(
                                tp[:, k * 128:(k + 1) * 128],
                                t_acc[:, b, k * 128:(k + 1) * 128],
                                t_ident[:])
                        nc.vector.tensor_tensor(
                            t_sp[:, n0:n0 + ns], tp[:, 0:ns],
                            t_st[:, n0:n0 + ns], OP.add)
                        nc.vector.tensor_tensor(
                            t_vp[:, :, n0:n0 + ns],
                            tp[:, 128:512].rearrange(
                                "p (d n) -> p d n", d=3)[:, :, 0:ns],
                            t_vfm[:, :, n0:n0 + ns], OP.add)
                    if debug:
                        nc.sync.dma_start(d_dbg_sp[:, :], t_sp[:])
                        nc.sync.dma_start(d_dbg_vp[:, :, :], t_vp[:])
                        nc.sync.dma_start(d_dbg_acc[:, :, :], t_acc[:])

            # ---------------- Phase N-B..H (st/vfm/acc freed)
            with tc.tile_pool(name="phn", bufs=2) as pool, \
                 tc.tile_pool(name="phnc", bufs=1) as pnc, \
                 tc.tile_pool(name="phn_ps", bufs=2, space="PSUM") as psp:
                t_wu = pnc.tile_from(d["wu"][:])
                t_bu = pnc.tile_from(d["buc"][:])
                t_wv = pnc.tile_from(d["wv"][:])
                t_bv = pnc.tile_from(d["bvc"][:])
                t_m1a = pnc.tile_from(d["m1a"][:])
                t_m1b = pnc.tile_from(d["m1b"][:])
                t_bm1 = pnc.tile_from(d["bm1c"][:])
                t_m2 = pnc.tile_from(d["m2"][:])
                t_bm2 = pnc.tile_from(d["bm2c"][:])
                t_uv = pnc.tile([F, 3, NPC], F32)
                t_vnorm = pnc.tile([F, NPC], F32)
                t_uvs = pnc.tile([F, NPC], F32)
                t_h = pnc.tile([F, NPC], F32)
                t_m = pnc.tile([F, 3, NPC], F32)
                ngrp = (NPC + NG - 1) // NG
                for g in range(ngrp):
                    n0 = g * NG
                    ns = min(NG, NPC - n0)
                    t_vv = pool.tile([F, 3, NG], F32, tag="vvg")
                    for dd in range(3):
                        psu = psp.tile([128, 512], F32, tag="u3")
                        nc.tensor.matmul(psu[:, :ns], t_wu[:],
                                         t_vp[:, dd, n0:n0 + ns],
                                         start=True, stop=True)
                        nc.scalar.activation(
                            t_uv[:, dd, n0:n0 + ns], psu[:, :ns],
                            AF.Identity, bias=t_bu[:], scale=1.0)
                        psvv = psp.tile([128, 512], F32, tag="u3")
                        nc.tensor.matmul(psvv[:, :ns], t_wv[:],
                                         t_vp[:, dd, n0:n0 + ns],
                                         start=True, stop=True)
                        nc.scalar.activation(
                            t_vv[:, dd, :ns], psvv[:, :ns],
                            AF.Identity, bias=t_bv[:], scale=1.0)
                    # vn2 = sum_d Vv^2 ; uvs = sum_d Uv*Vv
                    t_sq = pool.tile([F, 3, NG], F32, tag="sqg")
                    nc.vector.tensor_tensor(t_sq[:, :, :ns], t_vv[:, :, :ns],
                                            t_vv[:, :, :ns], OP.mult)
                    t_vn2 = pool.tile([F, NG], F32, tag="vn2")
                    nc.vector.tensor_reduce(
                        t_vn2[:, :ns],
                        t_sq[:, :, :ns].transpose((0, 2, 1)),
                        mybir.AxisListType.X, OP.add)
                    nc.vector.tensor_tensor(t_sq[:, :, :ns],
                                            t_uv[:, :, n0:n0 + ns],
                                            t_vv[:, :, :ns], OP.mult)
                    nc.vector.tensor_reduce(
                        t_uvs[:, n0:n0 + ns],
                        t_sq[:, :, :ns].transpose((0, 2, 1)),
                        mybir.AxisListType.X, OP.add)
                    nc.scalar.activation(t_vnorm[:, n0:n0 + ns],
                                         t_vn2[:, :ns], AF.Sqrt)
                    # h = silu(M1a@vnorm + M1b@s' + bm1)
                    psh = psp.tile([128, 512], F32, tag="h")
                    nc.tensor.matmul(psh[:, :ns], t_m1a[:],
                                     t_vnorm[:, n0:n0 + ns],
                                     start=True, stop=False)
                    nc.tensor.matmul(psh[:, :ns], t_m1b[:],
                                     t_sp[:, n0:n0 + ns],
                                     start=False, stop=True)
                    nc.scalar.activation(t_h[:, n0:n0 + ns], psh[:, :ns],
                                         AF.Silu, bias=t_bm1[:], scale=1.0)
                    # m = h@M2 + bm2  (3 chunks: avv, asv, ass)
                    for k in range(3):
                        psm = psp.tile([128, 512], F32, tag="m")
                        nc.tensor.matmul(psm[:, :ns],
                                         t_m2[:, k * F:(k + 1) * F],
                                         t_h[:, n0:n0 + ns],
                                         start=True, stop=True)
                        nc.scalar.activation(
                            t_m[:, k, n0:n0 + ns], psm[:, :ns],
                            AF.Identity, bias=t_bm2[:, k:k + 1], scale=1.0)

                # N-G: final combines
                nc.vector.tensor_tensor(
                    t_uv[:], t_uv[:],
                    t_m[:, 0:1, :].to_broadcast((F, 3, NPC)), OP.mult)
                nc.vector.tensor_tensor(t_vp[:], t_vp[:], t_uv[:], OP.add)
                nc.vector.tensor_tensor(t_uvs[:], t_uvs[:], t_m[:, 1, :],
                                        OP.mult)
                nc.vector.tensor_tensor(t_uvs[:], t_uvs[:], t_m[:, 2, :],
                                        OP.add)
                nc.vector.tensor_tensor(t_sp[:], t_sp[:], t_uvs[:], OP.add)

                # N-H: transpose back to node-major + DMA out
                for b in range(NBLK):
                    n0 = b * 128
                    ns = cfg.blk_rows(b)
                    tpo = psp.tile([128, 512], F32, tag="tp")
                    nc.tensor.transpose(tpo[:ns, 0:128],
                                        t_sp[:, n0:n0 + ns], t_ident[:])
                    for dd in range(3):
                        nc.tensor.transpose(
                            tpo[:ns, 128 + dd * 128:128 + (dd + 1) * 128],
                            t_vp[:, dd, n0:n0 + ns], t_ident[:])
                    t_so = pool.tile([128, F], F32, tag="so")
                    nc.scalar.activation(t_so[:ns, :], tpo[:ns, 0:F], AF.Copy)
                    nc.sync.dma_start(d_sout[n0:n0 + ns, :], t_so[:ns, :])
                    t_vo = pool.tile([128, F3], F32, tag="vo")
                    nc.vector.tensor_copy(
                        t_vo[:ns, :].rearrange("p (f d) -> p d f", d=3),
                        tpo[:ns, 128:512].rearrange("p (d f) -> p d f", d=3))
                    nc.sync.dma_start(d_vout[n0:n0 + ns, :], t_vo[:ns, :])

    nc.compile()
    return nc


# ------------------------------------------------------------------- runner

_CACHE = {}
LAST_RESULTS = None


def _install_ntff_hook():
    """Provide antenv.axon_hooks backed by /opt/axon/libaxon_pjrt.so so
    run_bass_kernel_spmd(trace=True) can capture NTFF profiles under axon."""
    import contextlib
    import ctypes
    import types

    if "antenv.axon_hooks" in sys.modules:
        return
    so_path = "/opt/axon/libaxon_pjrt.so"
    if not os.path.exists(so_path):
        return
    lib = ctypes.CDLL(so_path)
    if not hasattr(lib, "axon_start_nrt_profile"):
        return
    lib.axon_start_nrt_profile.argtypes = [
        ctypes.POINTER(ctypes.c_int64), ctypes.c_size_t]
    lib.axon_start_nrt_profile.restype = ctypes.c_int64
    lib.axon_stop_nrt_profile.argtypes = [ctypes.c_char_p]
    lib.axon_stop_nrt_profile.restype = ctypes.c_int64

    @contextlib.contextmanager
    def _hook(output_dir, device_ids):
        import jax
        jax.devices()
        if device_ids:
            ids = (ctypes.c_int64 * len(device_ids))(*device_ids)
            rc = lib.axon_start_nrt_profile(ids, len(device_ids))
        else:
            rc = lib.axon_start_nrt_profile(None, 0)
        if rc != 0:
            raise RuntimeError(f"axon_start_nrt_profile rc={rc}")
        try:
            yield
        finally:
            n = lib.axon_stop_nrt_profile(str(output_dir).encode())
            print(f"profile: {n} file(s) written to {output_dir}",
                  file=sys.stderr)

    _hook_holder = [_hook]
    mod = types.ModuleType("antenv.axon_hooks")
    mod.get_axon_ntff_profile_hook = lambda: _hook_holder[0]
    mod.set_axon_ntff_profile_hook = lambda h: _hook_holder.__setitem__(0, h)
    sys.modules["antenv.axon_hooks"] = mod


def run(inputs, cfg=CFG, sim=False, trace=False):
    global LAST_RESULTS
    in_maps, meta = host_prep(inputs, cfg)
    key = (cfg.N, cfg.E, cfg.C, meta["t_blk"])
    if key not in _CACHE:
        _CACHE[key] = build_program(cfg, meta["t_blk"])
    nc = _CACHE[key]

    if sim:
        from concourse.bass_interp import MultiCoreSim
        ms = MultiCoreSim(nc, num_cores=cfg.C, require_finite=False,
                          require_nnan=False)
        sims = list(ms.cores.values())
        for c, m in enumerate(in_maps):
            for k2, v2 in m.items():
                sims[c].tensor(k2)[:] = v2
        ms.simulate(check_with_hw=False)
        outs = [{"s_out": np.asarray(s_.tensor("s_out")),
                 "v_out": np.asarray(s_.tensor("v_out"))} for s_ in sims]
    else:
        from concourse.bass_utils import run_bass_kernel_spmd
        if trace:
            _install_ntff_hook()
        res = run_bass_kernel_spmd(nc, in_maps, core_ids=list(range(cfg.C)),
                                   trace=trace)
        LAST_RESULTS = res
        outs = res.results

    s_out = np.concatenate([o["s_out"] for o in outs], 0)
    v_out = np.concatenate([o["v_out"] for o in outs], 0)
    return s_out, v_out.reshape(cfg.N, cfg.F, 3)


def kernel(**inputs):
    s_out, v_out = run(inputs, CFG, sim=False,
                       trace=bool(int(os.environ.get("KERNEL_TRACE", "0"))))
    return (s_out, v_out)


# revision 11
# speedup vs baseline: 91.9677x; 1.1589x over previous
"""Trainium2 Bass kernel for nn_MessageBlock (PaiNN-style GNN message passing).

Strategy (8 cores, destination-sharded):
- Nodes are split 2500/core. Each core processes exactly the edges whose
  i_index lands in its node slice (host buckets edges by dest 128-node block,
  pads each block's edge list to a uniform tile count -> identical SPMD
  program on every core, no cross-core scatter traffic).
- Phase D: dense per-node phi2 = silu(s@W1+b1)@W2+b2 for the core's own
  nodes (feature-major matmuls), cast fp16, AllGather -> phi2_all in DRAM.
- Phase E: per 128-edge subtile: dma_gather phi2_all[j] and v[j] (fp16),
  RBF filter x = [rbf;1]@[Wr;br] (K=21 matmul), cutoff gate
  W = 1 - Sin(x*pi/10)^2  (== 0.5*(cos(x*pi/5)+1); the x<5 mask is provably
  always true for rbf in [0,1] with |Wr| <= 1/sqrt(20)), payload build on
  DVE in fp16, scatter-add via one-hot matmul into per-block PSUM
  accumulators [ds | dv(d-major)] in fp32.
- Phase N: PE-transpose deltas to feature-major, v'=v+dv, s'=s+ds, then the
  node update MLP (Uv/Vv matmuls, |Vv| via Sqrt, gated update), transpose
  back to node-major and DMA out.
"""

import os
import sys

sys.path.insert(0, "/opt/trn_rl_repo")

import numpy as np

import concourse.bass as bass
import concourse.bacc as bacc
import concourse.mybir as mybir
from concourse.tile import TileContext

F16 = mybir.dt.float16
F32 = mybir.dt.float32
I16 = mybir.dt.int16

PI = float(np.pi)


class Cfg:
    def __init__(self, n_nodes=20000, n_edges=320000, n_cores=8, nrbf=20, f=128):
        self.N = n_nodes
        self.E = n_edges
        self.C = n_cores
        self.NRBF = nrbf
        self.F = f
        assert n_nodes % n_cores == 0
        self.NPC = n_nodes // n_cores            # nodes per core
        self.NBLK = (self.NPC + 127) // 128      # dest blocks per core
        # node-group size for phase D/N matmuls (<=512, divides work evenly)
        self.NG = min(500, self.NPC)

    def blk_rows(self, b):
        return min(128, self.NPC - b * 128)


CFG = Cfg()


# ----------------------------------------------------------------- host prep

def host_prep(inputs, cfg):
    """Bucket/pad edges, build per-core arrays. Returns (in_maps, meta)."""
    N, E, C, F, NRBF = cfg.N, cfg.E, cfg.C, cfg.F, cfg.NRBF
    NPC, NBLK = cfg.NPC, cfg.NBLK

    s = np.asarray(inputs["s"], np.float32)
    v = np.asarray(inputs["v"], np.float32)
    rbf = np.asarray(inputs["rbf"], np.float32)
    dirv = np.asarray(inputs["r_ij_direction"], np.float32)
    i_idx = np.asarray(inputs["i_index"]).astype(np.int64)
    j_idx = np.asarray(inputs["j_index"]).astype(np.int64)

    # global bucket: core, then 128-node block within core
    core_of = i_idx // NPC
    blk_of = (i_idx % NPC) // 128
    order = np.lexsort((blk_of, core_of))
    # per (core, block) counts
    counts = np.zeros((C, NBLK), np.int64)
    np.add.at(counts, (core_of, blk_of), 1)
    t_blk = max(1, int(np.max((counts + 127) // 128)))
    pblk = t_blk * 128                      # padded edges per block
    epad = NBLK * pblk                      # padded edges per core
    tt = NBLK * t_blk                       # subtiles per core

    # scatter edges into padded slots
    j_pad = np.zeros((C, epad), np.int64)
    iloc_pad = np.full((C, epad), 300.0, np.float32)
    rbf_pad = np.zeros((C, epad, NRBF), np.float32)
    dir_pad = np.zeros((C, epad, 3), np.float32)

    e_sorted = order
    cs = core_of[e_sorted]
    bs = blk_of[e_sorted]
    # position within (core, block)
    # lexsorted -> edges of each (c,b) contiguous; rank within group:
    grp = cs * NBLK + bs
    grp_start = np.searchsorted(grp, np.arange(C * NBLK), side="left")
    pos_in_grp = np.arange(E) - grp_start[grp]
    slot = bs * pblk + pos_in_grp
    j_pad[cs, slot] = j_idx[e_sorted]
    iloc_pad[cs, slot] = (i_idx[e_sorted] % NPC - bs * 128).astype(np.float32)
    rbf_pad[cs, slot] = rbf[e_sorted]
    dir_pad[cs, slot] = dirv[e_sorted]

    # weights / constants (identical per core)
    def f32(x):
        return np.ascontiguousarray(np.asarray(x, np.float32))

    W1, b1 = f32(inputs["W1"]), f32(inputs["b1"])
    W2, b2 = f32(inputs["W2"]), f32(inputs["b2"])
    Wr, br = f32(inputs["Wr"]), f32(inputs["br"])
    WU, bU = f32(inputs["WU"]), f32(inputs["bU"])
    WV, bV = f32(inputs["WV"]), f32(inputs["bV"])
    M1, bm1 = f32(inputs["M1"]), f32(inputs["bm1"])
    M2, bm2 = f32(inputs["M2"]), f32(inputs["bm2"])

    wr_aug = np.concatenate([Wr, br[None, :]], 0)          # [NRBF+1, 3F]
    ident = np.eye(128, dtype=np.float32)
    iota16 = np.broadcast_to(
        np.arange(128, dtype=np.float32).astype(np.float16), (128, 128)).copy()
    ones_row = np.ones((1, 128), np.float32)
    v_dm = np.ascontiguousarray(
        v.transpose((0, 2, 1)).reshape(N, 3 * F)).astype(np.float16)  # [N, 3F] d-major

    common = dict(
        w1=W1, b1c=b1[:, None], w2=W2, b2r=b2[None, :],
        wr_aug=wr_aug, ones_row=ones_row,
        wu=WU, buc=bU[:, None], wv=WV, bvc=bV[:, None],
        m1a=np.ascontiguousarray(M1[:F]), m1b=np.ascontiguousarray(M1[F:]),
        bm1c=bm1[:, None],
        m2=M2, bm2c=np.ascontiguousarray(bm2.reshape(3, F).T),  # [F, 3]
        ident=ident, iota16=iota16,
    )

    in_maps = []
    for c in range(C):
        lo, hi = c * NPC, (c + 1) * NPC
        jw = np.tile(
            np.ascontiguousarray(j_pad[c].astype(np.int16).reshape(-1, 16).T),
            (8, 1))                                         # [128, epad/16]
        m = dict(common)
        m["v_dm"] = np.ascontiguousarray(v_dm[lo:hi])
        m.update(
            st_slice=np.ascontiguousarray(s[lo:hi].T),          # [F, NPC]
            v_fm=np.ascontiguousarray(v[lo:hi].transpose(2, 1, 0)),  # [3,F,NPC]->? see below
            jw=jw,
            iloc=np.ascontiguousarray(
                iloc_pad[c].reshape(tt, 128).T),                # [128, tt]
            dir_t=np.ascontiguousarray(
                dir_pad[c].reshape(tt, 128, 3).transpose(1, 0, 2)),  # [128, tt, 3]
            rbf_aug=np.ascontiguousarray(
                np.concatenate([rbf_pad[c],
                                np.ones((epad, 1), np.float32)], 1).T),  # [NRBF+1, epad]
        )
        # v_fm device layout [F, 3, NPC]
        m["v_fm"] = np.ascontiguousarray(v[lo:hi].transpose(1, 2, 0))
        in_maps.append(m)

    meta = dict(t_blk=t_blk, epad=epad, tt=tt)
    return in_maps, meta


# ------------------------------------------------------------- build program

def build_program(cfg, t_blk, debug=False):
    N, C, F, NRBF = cfg.N, cfg.C, cfg.F, cfg.NRBF
    NPC, NBLK, NG = cfg.NPC, cfg.NBLK, cfg.NG
    F3 = 3 * F
    pblk = t_blk * 128
    epad = NBLK * pblk
    tt = NBLK * t_blk

    nc = bacc.Bacc("TRN2", target_bir_lowering=False, num_devices=C,
                   num_swdge_queues=2)

    # ---- dram I/O
    d = {}
    def din(name, shape, dt=F32):
        d[name] = nc.dram_tensor(name, shape, dt, kind="ExternalInput")
        return d[name]

    din("w1", [F, F]); din("b1c", [F, 1])
    din("w2", [F, F3]); din("b2r", [1, F3])
    din("wr_aug", [NRBF + 1, F3]); din("ones_row", [1, 128])
    din("wu", [F, F]); din("buc", [F, 1])
    din("wv", [F, F]); din("bvc", [F, 1])
    din("m1a", [F, F]); din("m1b", [F, F]); din("bm1c", [F, 1])
    din("m2", [F, F3]); din("bm2c", [F, 3])
    din("ident", [128, 128]); din("iota16", [128, 128], F16)
    din("v_dm", [NPC, F3], F16)
    din("st_slice", [F, NPC]); din("v_fm", [F, 3, NPC])
    din("jw", [128, epad // 16], I16)
    din("iloc", [128, tt]); din("dir_t", [128, tt, 3])
    din("rbf_aug", [NRBF + 1, epad])

    d_sout = nc.dram_tensor("s_out", [NPC, F], F32, kind="ExternalOutput")
    d_vout = nc.dram_tensor("v_out", [NPC, F3], F32, kind="ExternalOutput")

    # collective tensors (internal dram)
    d_dbg_sp = d_dbg_vp = None
    if debug:
        d_dbg_sp = nc.dram_tensor("dbg_sp", [F, NPC], F32, kind="ExternalOutput")
        d_dbg_vp = nc.dram_tensor("dbg_vp", [F, 3, NPC], F32, kind="ExternalOutput")
        d_dbg_acc = nc.dram_tensor("dbg_acc", [128, NBLK, 512], F32, kind="ExternalOutput")
    d_comb_slice = nc.dram_tensor("comb_slice", [NPC, 2 * F3], F16,
                                  kind="Internal")
    d_comb_all = nc.dram_tensor(
        "comb_all", [N, 2 * F3], F16, kind="Internal", addr_space="Shared")

    AF = mybir.ActivationFunctionType
    OP = mybir.AluOpType

    with TileContext(nc) as tc:
        with tc.tile_pool(name="const", bufs=1) as cpool, \
             tc.tile_pool(name="spvp", bufs=1) as poolsv:
            t_ident = cpool.tile_from(d["ident"][:])
            t_sp = poolsv.tile([F, NPC], F32)          # s' feature-major
            t_vp = poolsv.tile([F, 3, NPC], F32)       # v' feature-major

            with tc.tile_pool(name="de", bufs=1) as poolde:
                t_st = poolde.tile_from(d["st_slice"][:])
                t_vfm = poolde.tile_from(d["v_fm"][:])
                t_acc = poolde.tile([128, NBLK, 512], F32)  # scatter accum

                # ---------------- Phase D: dense phi2 for own nodes
                with tc.tile_pool(name="phd", bufs=2) as pool, \
                     tc.tile_pool(name="phdc", bufs=1) as pdc, \
                     tc.tile_pool(name="phd_ps", bufs=2, space="PSUM") as psp:
                    t_w1 = pdc.tile_from(d["w1"][:])
                    t_b1 = pdc.tile_from(d["b1c"][:])
                    t_w2 = pdc.tile_from(d["w2"][:])
                    t_b2r = pdc.tile_from(d["b2r"][:])
                    t_ones = pdc.tile_from(d["ones_row"][:])
                    t_phi1 = pdc.tile([F, NPC], F32)
                    ngrp = (NPC + NG - 1) // NG
                    for g in range(ngrp):
                        n0 = g * NG
                        ns = min(NG, NPC - n0)
                        ps = psp.tile([128, 512], F32, tag="d1")
                        nc.tensor.matmul(ps[:, :ns], t_w1[:],
                                         t_st[:, n0:n0 + ns],
                                         start=True, stop=True)
                        nc.scalar.activation(t_phi1[:, n0:n0 + ns], ps[:, :ns],
                                             AF.Silu, bias=t_b1[:], scale=1.0)
                    for nb in range(NBLK):
                        n0 = nb * 128
                        ns = min(128, NPC - n0)
                        ps2 = psp.tile([128, F3], F32, tag="d2")
                        nc.tensor.matmul(ps2[:ns, :], t_phi1[:, n0:n0 + ns],
                                         t_w2[:], start=True, stop=False)
                        nc.tensor.matmul(ps2[:ns, :], t_ones[:1, :ns], t_b2r[:],
                                         start=False, stop=True)
                        t_p2 = pool.tile([128, F3], F16, tag="p2")
                        nc.scalar.activation(t_p2[:ns, :], ps2[:ns, :], AF.Copy)
                        nc.sync.dma_start(d_comb_slice[n0:n0 + ns, 0:F3],
                                          t_p2[:ns, :])
                    # v columns of the combined gather table
                    nc.sync.dma_start(d_comb_slice[:, F3:2 * F3], d["v_dm"][:])

                # AllGather phi2 slices -> phi2_all
                nc.gpsimd.collective_compute(
                    "AllGather", OP.bypass,
                    replica_groups=[list(range(C))],
                    ins=[d_comb_slice[:, :]],
                    outs=[d_comb_all[:, :]],
                )

                # ---------------- Phase E: edges
                GSZ = 4  # subtiles per group
                with tc.tile_pool(name="phe", bufs=2) as pool, \
                     tc.tile_pool(name="phec", bufs=1) as pec, \
                     tc.tile_pool(name="phe_x", bufs=1, space="PSUM") as pspx, \
                     tc.tile_pool(name="phe_a", bufs=2, space="PSUM") as pspa:
                    t_wr = pec.tile_from(d["wr_aug"][:])
                    t_iota = pec.tile_from(d["iota16"][:])
                    t_iloc = pec.tile_from(d["iloc"][:])
                    t_dirt = pec.tile_from(d["dir_t"][:])
                    t_jw = pec.tile_from(d["jw"][:])
                    for b in range(NBLK):
                        acc = pspa.tile([128, 512], F32, tag="acc")
                        st_base = b * t_blk
                        for g0 in range(0, t_blk, GSZ):
                            gsz = min(GSZ, t_blk - g0)
                            st0 = st_base + g0          # global subtile idx
                            ne = gsz * 128
                            e0 = st0 * 128
                            t_gc = pool.tile([128, GSZ, 2 * F3], F16,
                                             tag="gc")
                            nc.gpsimd.dma_gather(
                                t_gc[:, :gsz, :], d_comb_all[:, :],
                                t_jw[:, st0 * 8: st0 * 8 + gsz * 8],
                                ne, ne, 2 * F3, queue_num=b % 2)
                            t_phi = t_gc[:, :, 0:F3]
                            t_vj = t_gc[:, :, F3:2 * F3]
                            t_rbf = pool.tile([NRBF + 1, GSZ * 128], F32,
                                              tag="rbf")
                            nc.sync.dma_start(
                                t_rbf[:, :ne], d["rbf_aug"][:, e0:e0 + ne])
                            xps = pspx.tile([128, GSZ, 512], F32, tag="xps")
                            for si in range(gsz):
                                nc.tensor.matmul(
                                    xps[:, si, 0:F3],
                                    t_rbf[:, si * 128:(si + 1) * 128],
                                    t_wr[:], start=True, stop=True)
                            # gate: W = 1 - sin(x*pi/10)^2
                            t_u = pool.tile([128, GSZ, F3], F16, tag="u")
                            nc.scalar.activation(t_u[:, :gsz, :],
                                                 xps[:, :gsz, 0:F3],
                                                 AF.Sin, bias=0.0,
                                                 scale=PI / 10)
                            t_u2 = pool.tile([128, GSZ, F3], F16, tag="u2")
                            nc.scalar.activation(t_u2[:, :gsz, :],
                                                 t_u[:, :gsz, :], AF.Square)
                            # t_split = (u2 - 1) * phi = -(W * phi)
                            t_split = pool.tile([128, GSZ, F3], F16,
                                                tag="split")
                            nc.vector.scalar_tensor_tensor(
                                t_split[:, :gsz, :], t_u2[:, :gsz, :], 1.0,
                                t_phi[:, :gsz, :], OP.subtract, OP.mult)
                            t_dvp = pool.tile([128, GSZ, 3, F], F16, tag="dvp")
                            nc.vector.tensor_tensor(
                                t_dvp[:, :gsz, :, :],
                                t_split[:, :gsz, 2 * F:3 * F].unsqueeze(2)
                                    .to_broadcast((128, gsz, 3, F)),
                                t_vj[:, :gsz, :].rearrange(
                                    "p s (d f) -> p s d f", d=3),
                                OP.mult)
                            t_oh = pool.tile([128, GSZ, 128], F16, tag="oh")
                            nc.vector.tensor_tensor(
                                t_oh[:, :gsz, :],
                                t_iota[:].unsqueeze(1)
                                    .to_broadcast((128, gsz, 128)),
                                t_iloc[:, st0:st0 + gsz].unsqueeze(2)
                                    .to_broadcast((128, gsz, 128)),
                                OP.is_equal)
                            t_rd = pool.tile([128, GSZ, 3, F], F16, tag="rd")
                            nc.vector.tensor_tensor(
                                t_rd[:, :gsz, :, :],
                                t_split[:, :gsz, F:2 * F].unsqueeze(2)
                                    .to_broadcast((128, gsz, 3, F)),
                                t_dirt[:, st0:st0 + gsz, :].unsqueeze(3)
                                    .to_broadcast((128, gsz, 3, F)),
                                OP.mult)
                            for si in range(gsz):
                                first = (g0 == 0 and si == 0)
                                last = (g0 + gsz == t_blk and si == gsz - 1)
                                nc.tensor.matmul(
                                    acc[:, 0:F], t_oh[:, si, :],
                                    t_split[:, si, 0:F],
                                    start=first, stop=last,
                                    skip_group_check=True)
                                nc.tensor.matmul(
                                    acc[:, F:512], t_oh[:, si, :],
                                    t_dvp[:, si, :, :].rearrange(
                                        "p d f -> p (d f)"),
                                    start=False, stop=False,
                                    skip_group_check=True)
                                nc.tensor.matmul(
                                    acc[:, F:512], t_oh[:, si, :],
                                    t_rd[:, si, :, :].rearrange(
                                        "p d f -> p (d f)"),
                                    start=False, stop=last,
                                    skip_group_check=True)
                        nc.vector.tensor_copy(t_acc[:, b, :], acc[:])

                # ---------------- Phase N-A: transpose deltas, add bases
                with tc.tile_pool(name="phna_ps", bufs=2,
                                  space="PSUM") as psp:
                    for b in range(NBLK):
                        n0 = b * 128
                        ns = cfg.blk_rows(b)
                        tp = psp.tile([128, 512], F32, tag="tp")
                        for k in range(4):
                            nc.tensor.transpose(
                                tp[:, k * 128:(k + 1) * 128],
                                t_acc[:, b, k * 128:(k + 1) * 128],
                                t_ident[:])
                        nc.vector.tensor_tensor(
                            t_sp[:, n0:n0 + ns], t_st[:, n0:n0 + ns],
                            tp[:, 0:ns], OP.subtract)
                        nc.vector.tensor_tensor(
                            t_vp[:, :, n0:n0 + ns],
                            t_vfm[:, :, n0:n0 + ns],
                            tp[:, 128:512].rearrange(
                                "p (d n) -> p d n", d=3)[:, :, 0:ns],
                            OP.subtract)
                    if debug:
                        nc.sync.dma_start(d_dbg_sp[:, :], t_sp[:])
                        nc.sync.dma_start(d_dbg_vp[:, :, :], t_vp[:])
                        nc.sync.dma_start(d_dbg_acc[:, :, :], t_acc[:])

            # ---------------- Phase N-B..H (st/vfm/acc freed)
            with tc.tile_pool(name="phn", bufs=2) as pool, \
                 tc.tile_pool(name="phnc", bufs=1) as pnc, \
                 tc.tile_pool(name="phn_ps", bufs=2, space="PSUM") as psp:
                t_wu = pnc.tile_from(d["wu"][:])
                t_bu = pnc.tile_from(d["buc"][:])
                t_wv = pnc.tile_from(d["wv"][:])
                t_bv = pnc.tile_from(d["bvc"][:])
                t_m1a = pnc.tile_from(d["m1a"][:])
                t_m1b = pnc.tile_from(d["m1b"][:])
                t_bm1 = pnc.tile_from(d["bm1c"][:])
                t_m2 = pnc.tile_from(d["m2"][:])
                t_bm2 = pnc.tile_from(d["bm2c"][:])
                t_uv = pnc.tile([F, 3, NPC], F32)
                t_vnorm = pnc.tile([F, NPC], F32)
                t_uvs = pnc.tile([F, NPC], F32)
                t_h = pnc.tile([F, NPC], F32)
                t_m = pnc.tile([F, 3, NPC], F32)
                ngrp = (NPC + NG - 1) // NG
                for g in range(ngrp):
                    n0 = g * NG
                    ns = min(NG, NPC - n0)
                    t_vv = pool.tile([F, 3, NG], F32, tag="vvg")
                    for dd in range(3):
                        psu = psp.tile([128, 512], F32, tag="u3")
                        nc.tensor.matmul(psu[:, :ns], t_wu[:],
                                         t_vp[:, dd, n0:n0 + ns],
                                         start=True, stop=True)
                        nc.scalar.activation(
                            t_uv[:, dd, n0:n0 + ns], psu[:, :ns],
                            AF.Identity, bias=t_bu[:], scale=1.0)
                        psvv = psp.tile([128, 512], F32, tag="u3")
                        nc.tensor.matmul(psvv[:, :ns], t_wv[:],
                                         t_vp[:, dd, n0:n0 + ns],
                                         start=True, stop=True)
                        nc.scalar.activation(
                            t_vv[:, dd, :ns], psvv[:, :ns],
                            AF.Identity, bias=t_bv[:], scale=1.0)
                    # vn2 = sum_d Vv^2 ; uvs = sum_d Uv*Vv
                    t_sq = pool.tile([F, 3, NG], F32, tag="sqg")
                    nc.vector.tensor_tensor(t_sq[:, :, :ns], t_vv[:, :, :ns],
                                            t_vv[:, :, :ns], OP.mult)
                    t_vn2 = pool.tile([F, NG], F32, tag="vn2")
                    nc.vector.tensor_reduce(
                        t_vn2[:, :ns],
                        t_sq[:, :, :ns].transpose((0, 2, 1)),
                        mybir.AxisListType.X, OP.add)
                    nc.vector.tensor_tensor(t_sq[:, :, :ns],
                                            t_uv[:, :, n0:n0 + ns],
                                            t_vv[:, :, :ns], OP.mult)
                    nc.vector.tensor_reduce(
                        t_uvs[:, n0:n0 + ns],
                        t_sq[:, :, :ns].transpose((0, 2, 1)),
                        mybir.AxisListType.X, OP.add)
                    nc.scalar.activation(t_vnorm[:, n0:n0 + ns],
                                         t_vn2[:, :ns], AF.Sqrt)
                    # h = silu(M1a@vnorm + M1b@s' + bm1)
                    psh = psp.tile([128, 512], F32, tag="h")
                    nc.tensor.matmul(psh[:, :ns], t_m1a[:],
                                     t_vnorm[:, n0:n0 + ns],
                                     start=True, stop=False)
                    nc.tensor.matmul(psh[:, :ns], t_m1b[:],
                                     t_sp[:, n0:n0 + ns],
                                     start=False, stop=True)
                    nc.scalar.activation(t_h[:, n0:n0 + ns], psh[:, :ns],
                                         AF.Silu, bias=t_bm1[:], scale=1.0)
                    # m = h@M2 + bm2  (3 chunks: avv, asv, ass)
                    for k in range(3):
                        psm = psp.tile([128, 512], F32, tag="m")
                        nc.tensor.matmul(psm[:, :ns],
                                         t_m2[:, k * F:(k + 1) * F],
                                         t_h[:, n0:n0 + ns],
                                         start=True, stop=True)
                        nc.scalar.activation(
                            t_m[:, k, n0:n0 + ns], psm[:, :ns],
                            AF.Identity, bias=t_bm2[:, k:k + 1], scale=1.0)

                # N-G: final combines
                nc.vector.tensor_tensor(
                    t_uv[:], t_uv[:],
                    t_m[:, 0:1, :].to_broadcast((F, 3, NPC)), OP.mult)
                nc.vector.tensor_tensor(t_vp[:], t_vp[:], t_uv[:], OP.add)
                nc.vector.tensor_tensor(t_uvs[:], t_uvs[:], t_m[:, 1, :],
                                        OP.mult)
                nc.vector.tensor_tensor(t_uvs[:], t_uvs[:], t_m[:, 2, :],
                                        OP.add)
                nc.vector.tensor_tensor(t_sp[:], t_sp[:], t_uvs[:], OP.add)

                # N-H: transpose back to node-major + DMA out
                for b in range(NBLK):
                    n0 = b * 128
                    ns = cfg.blk_rows(b)
                    tpo = psp.tile([128, 512], F32, tag="tp")
                    nc.tensor.transpose(tpo[:ns, 0:128],
                                        t_sp[:, n0:n0 + ns], t_ident[:])
                    for dd in range(3):
                        nc.tensor.transpose(
                            tpo[:ns, 128 + dd * 128:128 + (dd + 1) * 128],
                            t_vp[:, dd, n0:n0 + ns], t_ident[:])
                    t_so = pool.tile([128, F], F32, tag="so")
                    nc.scalar.activation(t_so[:ns, :], tpo[:ns, 0:F], AF.Copy)
                    nc.sync.dma_start(d_sout[n0:n0 + ns, :], t_so[:ns, :])
                    t_vo = pool.tile([128, F3], F32, tag="vo")
                    nc.vector.tensor_copy(
                        t_vo[:ns, :].rearrange("p (f d) -> p d f", d=3),
                        tpo[:ns, 128:512].rearrange("p (d f) -> p d f", d=3))
                    nc.sync.dma_start(d_vout[n0:n0 + ns, :], t_vo[:ns, :])

    nc.compile()
    return nc


# ------------------------------------------------------------------- runner

_CACHE = {}
LAST_RESULTS = None


def _install_ntff_hook():
    """Provide antenv.axon_hooks backed by /opt/axon/libaxon_pjrt.so so
    run_bass_kernel_spmd(trace=True) can capture NTFF profiles under axon."""
    import contextlib
    import ctypes
    import types

    if "antenv.axon_hooks" in sys.modules:
        return
    so_path = "/opt/axon/libaxon_pjrt.so"
    if not os.path.exists(so_path):
        return
    lib = ctypes.CDLL(so_path)
    if not hasattr(lib, "axon_start_nrt_profile"):
        return
    lib.axon_start_nrt_profile.argtypes = [
        ctypes.POINTER(ctypes.c_int64), ctypes.c_size_t]
    lib.axon_start_nrt_profile.restype = ctypes.c_int64
    lib.axon_stop_nrt_profile.argtypes = [ctypes.c_char_p]
    lib.axon_stop_nrt_profile.restype = ctypes.c_int64

    @contextlib.contextmanager
    def _hook(output_dir, device_ids):
        import jax
        jax.devices()
        if device_ids:
            ids = (ctypes.c_int64 * len(device_ids))(*device_ids)
            rc = lib.axon_start_nrt_profile(ids, len(device_ids))
        else:
            rc = lib.axon_start_nrt_profile(None, 0)
        if rc != 0:
            raise RuntimeError(f"axon_start_nrt_profile rc={rc}")
        try:
            yield
        finally:
            n = lib.axon_stop_nrt_profile(str(output_dir).encode())
            print(f"profile: {n} file(s) written to {output_dir}",
                  file=sys.stderr)

    _hook_holder = [_hook]
    mod = types.ModuleType("antenv.axon_hooks")
    mod.get_axon_ntff_profile_hook = lambda: _hook_holder[0]
    mod.set_axon_ntff_profile_hook = lambda h: _hook_holder.__setitem__(0, h)
    sys.modules["antenv.axon_hooks"] = mod


def run(inputs, cfg=CFG, sim=False, trace=False):
    global LAST_RESULTS
    in_maps, meta = host_prep(inputs, cfg)
    key = (cfg.N, cfg.E, cfg.C, meta["t_blk"])
    if key not in _CACHE:
        _CACHE[key] = build_program(cfg, meta["t_blk"])
    nc = _CACHE[key]

    if sim:
        from concourse.bass_interp import MultiCoreSim
        ms = MultiCoreSim(nc, num_cores=cfg.C, require_finite=False,
                          require_nnan=False)
        sims = list(ms.cores.values())
        for c, m in enumerate(in_maps):
            for k2, v2 in m.items():
                sims[c].tensor(k2)[:] = v2
        ms.simulate(check_with_hw=False)
        outs = [{"s_out": np.asarray(s_.tensor("s_out")),
                 "v_out": np.asarray(s_.tensor("v_out"))} for s_ in sims]
    else:
        from concourse.bass_utils import run_bass_kernel_spmd
        if trace:
            _install_ntff_hook()
        res = run_bass_kernel_spmd(nc, in_maps, core_ids=list(range(cfg.C)),
                                   trace=trace)
        LAST_RESULTS = res
        outs = res.results

    s_out = np.concatenate([o["s_out"] for o in outs], 0)
    v_out = np.concatenate([o["v_out"] for o in outs], 0)
    return s_out, v_out.reshape(cfg.N, cfg.F, 3)


def kernel(**inputs):
    s_out, v_out = run(inputs, CFG, sim=False,
                       trace=bool(int(os.environ.get("KERNEL_TRACE", "0"))))
    return (s_out, v_out)
